# revision 81
# baseline (speedup 1.0000x reference)
"""MultiHeadCrossAttention on 8 TRN2 NeuronCores.

Sharding: 2D (batch x head-group): core c handles batch b=c//2 and heads
hg*8..hg*8+7 (hg=c%2). Host sums the two head-group partials per batch.

Projections run as fp8e4 DoubleRow matmuls (0.5 cyc/row, 2 k-tiles per
instruction) with an error-feedback decomposition that keeps accuracy at
~2.5e-3: x = x_hi + x_lo (lo in fp8 subnormal range), W' = 16W = W_hi +
W_lo, and x@W'.T ~= x_hi@W_hi + x_lo@W_hi + x_hi@W_lo (the lo*lo term is
negligible). The /16 is folded into the PSUM->SBUF bias step. Attention
(QK^T, exp, PV, out-proj) runs in fp16 (full PE rate at any ap size).

Per core:
  Q^T [512,1024], K^T [512,2048] (dims on partitions), V [2048,512]
  computed directly (kv on partitions -> no V transpose); PV uses a
  ones-column in V's tile so the softmax denominator falls out of the
  same matmul; normalization is a per-partition tensor_scalar_mul.
  attn^T via PE transpose; Y^T = Wo^T.T @ attn^T, fp16 out.
bv/bo are folded on the host (y += bo + Wo@bv).

The q dim is processed in four 256-col blocks (u); attention is emitted
as a single rolling pipeline over 128 (u,h,kv-group) slots: QKT+exp at
slot i, PV at slot i+2, with projection/out-proj blocks dispensed from a
deadline-tracked fill queue into the ACT-gated gaps so the PE never
idles.
"""
import numpy as np
import ml_dtypes
from contextlib import ExitStack

import concourse.bass as bass
import concourse.mybir as mybir
import concourse.tile as tile
from concourse import bacc
from concourse.bass_utils import run_bass_kernel_spmd

N_CORES = 8
B, SQ, SKV, E = 4, 1024, 2048, 1024
H_PER, DC, EC, ECP, KVC, QU = 8, 4, 8, 4, 16, 4
F16 = mybir.dt.float16
F32 = mybir.dt.float32
F8 = mybir.dt.float8e4
NPF8 = ml_dtypes.float8_e4m3
Exp = mybir.ActivationFunctionType.Exp
DR = mybir.MatmulPerfMode.DoubleRow
MUL = mybir.AluOpType.mult
ADD = mybir.AluOpType.add

_CACHE = {}
PHASE_LOG = []


def _mark(nc, label):
    PHASE_LOG.append((int(nc.get_next_instruction_name().split("-")[1]), label))


def _build():
    nc = bacc.Bacc("TRN2", target_bir_lowering=False, debug=False,
                   num_devices=N_CORES)
    x1h = nc.dram_tensor("x1h", [128, QU, EC, 256], F8,
                         kind="ExternalInput").ap()
    x1l = nc.dram_tensor("x1l", [128, QU, EC, 256], F8,
                         kind="ExternalInput").ap()
    x2h = nc.dram_tensor("x2h", [128, 4, EC, 512], F8,
                         kind="ExternalInput").ap()
    x2l = nc.dram_tensor("x2l", [128, 4, EC, 512], F8,
                         kind="ExternalInput").ap()
    wqh = nc.dram_tensor("wqh", [128, EC, DC, 128], F8,
                         kind="ExternalInput").ap()
    wql = nc.dram_tensor("wql", [128, EC, DC, 128], F8,
                         kind="ExternalInput").ap()
    wkh = nc.dram_tensor("wkh", [128, EC, DC, 128], F8,
                         kind="ExternalInput").ap()
    wkl = nc.dram_tensor("wkl", [128, EC, DC, 128], F8,
                         kind="ExternalInput").ap()
    wvh = nc.dram_tensor("wvh", [128, EC, 512], F8, kind="ExternalInput").ap()
    wvl = nc.dram_tensor("wvl", [128, EC, 512], F8, kind="ExternalInput").ap()
    wot = nc.dram_tensor("wot", [128, DC, 8, 128], F16,
                         kind="ExternalInput").ap()
    bqv = nc.dram_tensor("bq", [128, DC], F32, kind="ExternalInput").ap()
    bkv = nc.dram_tensor("bk", [128, DC], F32, kind="ExternalInput").ap()
    idv = nc.dram_tensor("ident", [128, 128], F16, kind="ExternalInput").ap()
    onv = nc.dram_tensor("ones", [128, 1], F16, kind="ExternalInput").ap()
    yt = nc.dram_tensor("yt", [128, QU, 8, 256], F16,
                        kind="ExternalOutput").ap()

    with tile.TileContext(nc) as tc, ExitStack() as ctx:
        const = ctx.enter_context(tc.tile_pool(name="const", bufs=1))
        persist = ctx.enter_context(tc.tile_pool(name="persist", bufs=1))
        work = ctx.enter_context(tc.tile_pool(name="work", bufs=3))
        ps_pj = ctx.enter_context(tc.tile_pool(name="ps_pj", bufs=2,
                                               space="PSUM"))
        ps_s = ctx.enter_context(tc.tile_pool(name="ps_s", bufs=2,
                                              space="PSUM"))
        ps_o = ctx.enter_context(tc.tile_pool(name="ps_o", bufs=2,
                                              space="PSUM"))

        wqh_sb = const.tile([128, EC, DC, 128], F8, tag="wqh")
        wql_sb = const.tile([128, EC, DC, 128], F8, tag="wql")
        wkh_sb = const.tile([128, EC, DC, 128], F8, tag="wkh")
        wkl_sb = const.tile([128, EC, DC, 128], F8, tag="wkl")
        wvh_sb = const.tile([128, EC, 512], F8, tag="wvh")
        wvl_sb = const.tile([128, EC, 512], F8, tag="wvl")
        wo_sb = const.tile([128, DC, 8, 128], F16, tag="wo")
        bq_sb = const.tile([128, DC], F32, tag="bq")
        bk_sb = const.tile([128, DC], F32, tag="bk")
        id_sb = const.tile([128, 128], F16, tag="id")
        ones_sb = const.tile([128, 1], F16, tag="ones1")

        x1h_sb = persist.tile([128, QU, EC, 256], F8, tag="x1h")
        x1l_sb = persist.tile([128, QU, EC, 256], F8, tag="x1l")
        x2h_sb = persist.tile([128, 4, EC, 512], F8, tag="x2h")
        x2l_sb = persist.tile([128, 4, EC, 512], F8, tag="x2l")
        qt_sb = persist.tile([128, DC, SQ], F16, tag="qt")
        kt_sb = persist.tile([128, DC, SKV], F16, tag="kt")
        v_sb = persist.tile([128, KVC, 8 * 65], F16, tag="v")
        at_sb = persist.tile([128, DC, SQ], F16, tag="attnT")

        # arrival order tracks the front emission order: K(0,0) -> V(0..3)
        # -> Q(0,0) -> x2-chunk-paced slots
        nc.sync.dma_start(ones_sb[:], onv[:])
        for h2 in range(2):
            e0, e1 = 4 * h2, 4 * h2 + 4
            nc.sync.dma_start(wkh_sb[:, e0:e1], wkh[:, e0:e1])
            nc.sync.dma_start(x2h_sb[:, 0, e0:e1], x2h[:, 0, e0:e1])
            nc.sync.dma_start(x2l_sb[:, 0, e0:e1], x2l[:, 0, e0:e1])
            nc.sync.dma_start(wkl_sb[:, e0:e1], wkl[:, e0:e1])
        nc.sync.dma_start(wqh_sb[:], wqh[:])
        nc.sync.dma_start(x1h_sb[:, 0], x1h[:, 0])
        nc.sync.dma_start(x1l_sb[:, 0], x1l[:, 0])
        nc.sync.dma_start(wql_sb[:], wql[:])
        nc.sync.dma_start(bq_sb[:], bqv[:])
        nc.sync.dma_start(bk_sb[:], bkv[:])
        nc.sync.dma_start(wvh_sb[:], wvh[:])
        nc.sync.dma_start(wvl_sb[:], wvl[:])
        for j in range(1, 4):
            nc.sync.dma_start(x2h_sb[:, j], x2h[:, j])
            nc.sync.dma_start(x2l_sb[:, j], x2l[:, j])
        for u in range(1, QU):
            nc.sync.dma_start(x1h_sb[:, u], x1h[:, u])
            nc.sync.dma_start(x1l_sb[:, u], x1l[:, u])
        nc.sync.dma_start(wo_sb[:], wot[:])
        nc.sync.dma_start(id_sb[:], idv[:])

        def dr_terms(wh, wl, xh, xl):
            return ((wh, xh), (wh, xl), (wl, xh))

        def _dr_steps(tile_args, lhs_of, rhs_of, n, fin, cost):
            # micro-op list: one DoubleRow matmul per step, then a finisher
            st = {}

            def step(i):
                if i == 0:
                    st["ps"] = ps_pj.tile(*tile_args["shape"],
                                          tag="pj", name=tile_args["name"])
                nc.tensor.matmul(st["ps"][:], lhs_of(i), rhs_of(i),
                                 start=(i == 0), stop=(i == n - 1),
                                 perf_mode=DR)
                if i == n - 1:
                    fin(st["ps"])
            return [(cost, lambda i=i: step(i)) for i in range(n)]

        def proj_q(dc, u):
            terms = dr_terms(wqh_sb, wql_sb, x1h_sb, x1l_sb)

            def fin(ps):
                nc.vector.tensor_scalar(
                    qt_sb[:, dc, u * 256:(u + 1) * 256], ps[:],
                    1.0 / 16.0, bq_sb[:, dc:dc + 1], MUL, ADD)
            return _dr_steps(
                {"shape": ([128, 256], F32), "name": f"qps{dc}_{u}"},
                lambda i: terms[i // ECP][0][:, 2 * (i % ECP):2 * (i % ECP) + 2,
                                             dc],
                lambda i: terms[i // ECP][1][:, u, 2 * (i % ECP):
                                             2 * (i % ECP) + 2],
                3 * ECP, fin, 55)

        def proj_k(dc, j):
            terms = dr_terms(wkh_sb, wkl_sb, x2h_sb, x2l_sb)

            def fin(ps):
                nc.vector.tensor_scalar(
                    kt_sb[:, dc, j * 512:(j + 1) * 512], ps[:],
                    1.0 / 16.0, bk_sb[:, dc:dc + 1], MUL, ADD)
            return _dr_steps(
                {"shape": ([128, 512], F32), "name": f"kps{dc}_{j}"},
                lambda i: terms[i // ECP][0][:, 2 * (i % ECP):2 * (i % ECP) + 2,
                                             dc],
                lambda i: terms[i // ECP][1][:, j, 2 * (i % ECP):
                                             2 * (i % ECP) + 2],
                3 * ECP, fin, 110)

        def proj_v(kc):
            terms = ((x2h_sb, wvh_sb), (x2l_sb, wvh_sb), (x2h_sb, wvl_sb))

            def fin(ps):
                dst = v_sb[:, kc].rearrange("p (h x) -> p h x", x=65)
                nc.vector.tensor_scalar(
                    dst[:, :, 0:64],
                    ps[:].rearrange("p (h x) -> p h x", x=64),
                    1.0 / 16.0, None, MUL)
                last_v[0] = kc
            return _dr_steps(
                {"shape": ([128, 512], F32), "name": f"vps{kc}"},
                lambda i: terms[i // ECP][0][:, kc // 4,
                                             2 * (i % ECP):2 * (i % ECP) + 2,
                                             (kc % 4) * 128:(kc % 4 + 1) * 128],
                lambda i: terms[i // ECP][1][:, 2 * (i % ECP):
                                             2 * (i % ECP) + 2],
                3 * ECP, fin, 110)

        def fill_ones():
            vv = v_sb[:].rearrange("p kc (h x) -> p (kc h) x", x=65)
            nc.gpsimd.tensor_copy(
                vv[:, :, 64:65],
                ones_sb[:].unsqueeze(-1).to_broadcast((128, KVC * 8, 1)))

        y_sbs = [None] * QU

        def oproj(u, oc):
            st = {}

            def step(dcx):
                if dcx == 0:
                    st["ps"] = ps_pj.tile([128, 256], F32, tag="pj",
                                          name=f"yps{u}_{oc}")
                    if oc == 0:
                        y_sbs[u] = work.tile([128, 8, 256], F16, tag="y",
                                             bufs=2, name=f"ysb{u}")
                nc.tensor.matmul(st["ps"][:], wo_sb[:, dcx, oc],
                                 at_sb[:, dcx, u * 256:(u + 1) * 256],
                                 start=(dcx == 0), stop=(dcx == DC - 1))
                if dcx == DC - 1:
                    nc.vector.tensor_copy(y_sbs[u][:, oc], st["ps"][:])
                    if oc == 3:
                        nc.sync.dma_start(yt[:, u, 0:4], y_sbs[u][:, 0:4])
                    elif oc == 7:
                        nc.sync.dma_start(yt[:, u, 4:8], y_sbs[u][:, 4:8])
            return [(110, lambda d=d: step(d)) for d in range(DC)]

        def transp(u, dcx):
            def step(qc):
                tp = ps_pj.tile([128, 128], F16, tag="pj",
                                name=f"tp{u}_{qc}_{dcx}")
                nc.tensor.transpose(
                    tp[:], attn_sbs[u][:, qc, dcx * 128:(dcx + 1) * 128],
                    id_sb[:])
                nc.vector.tensor_copy(
                    at_sb[:, dcx,
                          u * 256 + qc * 128:u * 256 + (qc + 1) * 128],
                    tp[:])
            return [(60, lambda q=q: step(q)) for q in range(2)]

        # ---- rolling attention pipeline ----
        slots = [(u, h, g) for u in range(QU) for h in range(H_PER)
                 for g in range(4)]
        LAG = 2
        o_tiles = {}
        pt_tiles = {}
        attn_sbs = [None] * QU
        fill = []      # entries: (cost_ns, thunk)
        need = {}
        emitted = [0]
        fillB = []
        needB = {}
        emittedB = [0]
        ration_ns = [0.0]

        def drain_to(k):
            while emitted[0] < min(k, len(fill)):
                fill[emitted[0]][1]()
                emitted[0] += 1

        def drainB_to(k):
            while emittedB[0] < min(k, len(fillB)):
                cost, thunk = fillB[emittedB[0]]
                thunk()
                ration_ns[0] += cost
                emittedB[0] += 1

        def ration_fill(budget_ns):
            # pop gap-filler micros while behind the schedule
            while emittedB[0] < len(fillB) and ration_ns[0] < budget_ns:
                cost, thunk = fillB[emittedB[0]]
                thunk()
                ration_ns[0] += cost
                emittedB[0] += 1

        def emit_qkt(u, h, g):
            dc, hp = h // 2, (h % 2) * 64
            if g == 0:
                o_tiles[(u, h)] = ps_o.tile([128, 2, 65], F32, tag="o",
                                            name=f"ops{u}_{h}")
            s_ps = ps_s.tile([128, 4, 256], F32, tag="s",
                             name=f"sps{u}_{h}_{g}")
            for t in range(4):
                kc = 4 * g + t
                nc.tensor.matmul(
                    s_ps[:, t],
                    kt_sb[hp:hp + 64, dc, kc * 128:(kc + 1) * 128],
                    qt_sb[hp:hp + 64, dc, u * 256:(u + 1) * 256],
                    start=(t % 2 == 0), stop=(t % 2 == 1),
                    skip_group_check=True)
            pt = work.tile([128, 4, 256], F16, tag="pt", bufs=12,
                           name=f"pt{u}_{h}_{g}")
            nc.scalar.activation(pt[:], s_ps[:], Exp, scale=0.125)
            pt_tiles[(u, h, g)] = pt

        def emit_pv(u, h, g):
            pt = pt_tiles.pop((u, h, g))
            o_ps = o_tiles[(u, h)]
            for t in range(4):
                kc = 4 * g + t
                for qc in range(2):
                    nc.tensor.matmul(
                        o_ps[:, qc],
                        pt[:, t, qc * 128:(qc + 1) * 128],
                        v_sb[:, kc, h * 65:h * 65 + 65],
                        start=(kc == 0 and qc == 0),
                        stop=(kc == KVC - 1 and qc == 1),
                        skip_group_check=True)
            if g == 3:
                emit_norm(u, h)

        def emit_norm(u, h):
            o_ps = o_tiles.pop((u, h))
            for qc in range(2):
                recip = work.tile([128, 1], F32, tag="rc", bufs=4,
                                  name=f"rc{u}_{h}_{qc}")
                nc.vector.reciprocal(recip[:], o_ps[:, qc, 64:65])
                nc.vector.tensor_scalar_mul(
                    attn_sbs[u][:, qc, h * 64:(h + 1) * 64],
                    o_ps[:, qc, 0:64], recip[:])
            if h % 2 == 1:
                # heads 2*dcx, 2*dcx+1 done: their attn^T block is ready
                fillB.extend(blockify(transp(u, h // 2)))
            if h == H_PER - 1:
                for oc in range(8):
                    fillB.extend(blockify(oproj(u, oc)))

        # Two fill queues of single-instruction micros:
        #  - fillA: x2-chunk-paced work (K, V) emitted strictly by deadline;
        #    emitting it early would park a stalled instruction at the head
        #    of the in-order PE queue.
        #  - fillB: independently-gated work (Q, later O/transpose) that the
        #    ration pulls into any gap.
        # Slot of (u, h): half=h//4: 64*half + 16*u + 8*((h%4)//2)
        last_v = [-1]

        def blockify(micros):
            # emit a projection block contiguously: its PSUM-pool buffer is
            # then held only ~1 slot, avoiding head-of-line stalls when a
            # third pool tag-buffer would otherwise be requested mid-block
            total = sum(c for c, _ in micros)

            def run(ms=tuple(micros)):
                for _, th in ms:
                    th()
            return [(total, run)]

        blocksA = [(0, proj_v(0)), (0, proj_v(1)), (0, proj_k(0, 1)),
                   (1, proj_v(2)), (1, proj_v(3)), (1, proj_k(0, 2)),
                   (2, proj_k(0, 3)), (3, proj_v(4)), (4, proj_v(5))]
        blocksA += [(kc - 1, proj_v(kc)) for kc in range(6, KVC)]
        blocksA += [(5 + j, proj_k(1, j)) for j in range(4)]
        blocksA += [(13 + j, proj_k(2, j)) for j in range(4)]
        blocksA += [(21 + j, proj_k(3, j)) for j in range(4)]
        for dl, micros in sorted(blocksA, key=lambda b: b[0]):
            fill.extend(blockify(micros))
            need[dl] = max(need.get(dl, 0), len(fill))

        blocksB = [(5, proj_q(1, 0)), (13, proj_q(2, 0)),
                   (21, proj_q(3, 0))]
        blocksB += [(32 * u + 8 * dcb - 3, proj_q(dcb, u))
                    for u in range(1, QU) for dcb in range(DC)]
        for dl, micros in sorted(blocksB, key=lambda b: b[0]):
            fillB.extend(blockify(micros))
            needB[dl] = max(needB.get(dl, 0), len(fillB))

        # front: K(0,0) then Q(0,0) (their inputs are the first DMAs)
        for _, th in proj_k(0, 0):
            th()
        for _, th in proj_q(0, 0):
            th()
        fill_ones()

        # PV emission trails QKT through a dependency-checked pending queue:
        # a PV needs its exp LAG slots back, its V blocks emitted, and
        # per-head kc order.
        pv_pending = []

        def pv_ok(j, i):
            if j > i - LAG:
                return False
            return 4 * slots[j][2] + 3 <= last_v[0]

        def drain_pv(i, nmax):
            done = 0
            k = 0
            while k < len(pv_pending) and done < nmax:
                j = pv_pending[k]
                if pv_ok(j, i):
                    emit_pv(*slots[j])
                    pv_pending.pop(k)
                    done += 1
                else:
                    k += 1

        GAP = 380
        for i, (u, h, g) in enumerate(slots):
            _mark(nc, f"slot{i}_u{u}h{h}g{g}")
            if g == 0 and attn_sbs[u] is None:
                attn_sbs[u] = work.tile(
                    [128, 2, 512], F16, tag="attn", bufs=4, name=f"attn{u}")
            emit_qkt(u, h, g)
            pv_pending.append(i)
            drain_pv(i, 4 if len(pv_pending) > 6 else 2)
            ration_fill(60 * (i + 1))
            if i in need:
                drain_to(need[i])
            if i in needB:
                drainB_to(needB[i])
        _mark(nc, "tail")
        drain_pv(10 ** 9, 10 ** 9)
        drain_to(len(fill))
        drainB_to(len(fillB))
        _mark(nc, "end")

    nc.compile()
    return nc


def _get_nc():
    if "nc" not in _CACHE:
        _CACHE["nc"] = _build()
    return _CACHE["nc"]


def _split8(a):
    hi = a.astype(NPF8)
    lo = (a - hi.astype(np.float32)).astype(NPF8)
    return hi, lo


def make_in_maps(x1, x2, Wq, bq, Wk, bk, Wv, bv, Wo, bo=None):
    x1 = np.asarray(x1, np.float32)
    x2 = np.asarray(x2, np.float32)
    Wq = np.asarray(Wq, np.float32)
    Wk = np.asarray(Wk, np.float32)
    Wv = np.asarray(Wv, np.float32)
    Wo = np.asarray(Wo, np.float32)
    bq = np.asarray(bq, np.float32)
    bk = np.asarray(bk, np.float32)
    ident = np.eye(128, dtype=np.float16)
    ones = np.ones((128, 1), dtype=np.float16)

    def tile_x(xb, chunk):  # [S, E] fp8 -> [128, S/chunk, EC, chunk]
        return np.ascontiguousarray(
            xb.T.reshape(EC, 128, -1, chunk).transpose(1, 2, 0, 3))

    def tile_wqk(Ws):  # [512, E] fp8 -> [128, EC, DC, 128]
        return np.ascontiguousarray(
            Ws.reshape(DC, 128, EC, 128).transpose(3, 2, 0, 1))

    x1s = [tuple(tile_x(p, 256) for p in _split8(x1[b])) for b in range(B)]
    x2s = [tuple(tile_x(p, 512) for p in _split8(x2[b])) for b in range(B)]
    in_maps = []
    for c in range(N_CORES):
        b, hg = c // 2, c % 2
        s = slice(512 * hg, 512 * (hg + 1))
        qh, ql = _split8(16.0 * Wq[s, :])
        kh, kl = _split8(16.0 * Wk[s, :])
        vh, vl = _split8(16.0 * Wv[s, :])

        def tile_wv(Ws):  # [512, E] fp8 -> [128, EC, 512]
            return np.ascontiguousarray(Ws.reshape(512, EC, 128)
                                        .transpose(2, 1, 0))

        in_maps.append({
            "x1h": x1s[b][0], "x1l": x1s[b][1],
            "x2h": x2s[b][0], "x2l": x2s[b][1],
            "wqh": tile_wqk(qh), "wql": tile_wqk(ql),
            "wkh": tile_wqk(kh), "wkl": tile_wqk(kl),
            "wvh": tile_wv(vh), "wvl": tile_wv(vl),
            "wot": np.ascontiguousarray(
                Wo[:, s].reshape(8, 128, DC, 128).transpose(3, 2, 0, 1)
            ).astype(np.float16),
            "bq": np.ascontiguousarray(bq[s].reshape(DC, 128).T),
            "bk": np.ascontiguousarray(bk[s].reshape(DC, 128).T),
            "ident": ident, "ones": ones,
        })
    return in_maps


def kernel(x1, x2, Wq, bq, Wk, bk, Wv, bv, Wo, bo):
    nc = _get_nc()
    in_maps = make_in_maps(x1, x2, Wq, bq, Wk, bk, Wv, bv, Wo)
    res = run_bass_kernel_spmd(nc, in_maps, list(range(N_CORES)))
    y = np.zeros((B, SQ, E), np.float64)
    for c in range(N_CORES):
        ytc = res.results[c]["yt"].astype(np.float64)  # [128, QU, 8, 256]
        y[c // 2] += ytc.transpose(2, 0, 1, 3).reshape(E, SQ).T
    bo_eff = (np.asarray(bo, np.float64)
              + np.asarray(Wo, np.float64) @ np.asarray(bv, np.float64))
    return (y + bo_eff[None, None, :]).astype(np.float32)
